# revision 17
# baseline (speedup 1.0000x reference)
"""GAT (3-layer, no-LeakyReLU) on 8 Trainium2 NeuronCores.

Math: the reference omits LeakyReLU on attention logits, so softmax is
separable: with aj[n,h] = <h[n,h,:], att[h, C:]> and u = exp(aj),
    out[d] = sum_{e: dst=d} u[src_e] * h[src_e] / sum_e u[src_e]
(the ai[dst] term cancels inside the per-destination softmax).

Per layer, per core (nodes sharded by destination, graph structure
preprocessed on host):
  1. dense:  h = x @ W.T (rows layout, lhsT = transposed input xT)
             aj = x @ wj  (wj = W.T @ att_j, host-folded), u = exp(aj)
  2. G = [u*h | u | pad] bf16, AllGather -> replicated table in HBM
  3. per dst-block of 128 nodes: gather G[src] rows (dma_gather),
     one-hot matmul accumulates F = sum onehot.T @ g  and S1 = sum u
  4. out = F / S1 ; transpose to xT for the next layer's dense.
"""

import numpy as np
import ml_dtypes

N = 20000
E = 320000
NCORES = 8
NSH = 2500            # nodes per core (true)
NSHP = 2560           # padded to 20 x 128
NPAD = NCORES * NSHP  # padded table rows
P = 128
RT = NSHP // P        # row tiles per core = 20
NBLK = RT             # dst blocks per core = 20
GB = 8                # gather batch: tiles per dma_gather

# layer configs: (heads, C_out, KB in-blocks, G width, u col offset)
L1 = dict(H=4, C=128, KB=1, GW=640, UO=512, NOUT=512)
L2 = dict(H=4, C=128, KB=4, GW=640, UO=512, NOUT=512)
L3 = dict(H=1, C=100, KB=4, GW=256, UO=100, NOUT=100)

BF16 = ml_dtypes.bfloat16


NCH = 1               # source chunks (table split for AG/gather overlap)
CHR = NSHP // NCH     # 640 local rows per chunk


def _preprocess_edges(edge_index):
    """Sort edges by (dst core, src chunk, dst block); equalize per
    (chunk, block) tile counts across cores; build per-core gather
    indices (chunk-local) + one-hot tiles. Tiles are ordered pass-major
    (chunk outer, block inner) and gather batches never cross a chunk
    boundary, so pass-c gathers only need AG chunk c."""
    src = edge_index[0].astype(np.int64)
    dst = edge_index[1].astype(np.int64)
    core = dst // NSH
    d_loc = dst % NSH
    blk = d_loc // P
    d_in_blk = d_loc % P
    s_loc = src % NSH
    s_chunk = s_loc // CHR       # 0..3 (s_loc < 2500 < 2560)

    order = np.lexsort((src, blk, s_chunk, core))
    src_s = src[order]
    core_s = core[order]
    blk_s = blk[order]
    dib_s = d_in_blk[order]
    ch_s = s_chunk[order]

    # per (core, chunk, block) counts
    cnt = np.zeros((NCORES, NCH, NBLK), np.int64)
    for k in range(NCORES):
        m = core_s == k
        idx2 = ch_s[m] * NBLK + blk_s[m]
        cnt[k] = np.bincount(idx2, minlength=NCH * NBLK).reshape(NCH, NBLK)

    tb = np.ceil(cnt / P).astype(np.int64).max(axis=0)    # [NCH, NBLK]
    tch = tb.sum(axis=1)                                  # tiles per chunk
    T = int(tb.sum())
    nb_ch = [(int(t) + GB - 1) // GB for t in tch]        # batches per chunk
    NB = int(sum(nb_ch))

    # tile grid: pass-major (chunk, block, tile)
    chunk_of_tile = []
    block_of_tile = []
    first = []
    last = []
    for c in range(NCH):
        for b in range(NBLK):
            for t in range(tb[c, b]):
                chunk_of_tile.append(c)
                block_of_tile.append(b)
                first.append(t == 0)
                last.append(t == tb[c, b] - 1)
    chunk_of_tile = np.array(chunk_of_tile)
    block_of_tile = np.array(block_of_tile)
    first = np.array(first)
    last = np.array(last)

    # batch layout: per chunk, batches of GB tiles (last partial)
    batches = []   # (chunk, tile_start, n_tiles)
    tile_base = np.concatenate([[0], np.cumsum(tch)])
    for c in range(NCH):
        t0 = int(tile_base[c])
        for g in range(nb_ch[c]):
            s0 = t0 + g * GB
            batches.append((c, s0, min(GB, t0 + int(tch[c]) - s0)))

    # slot offsets of each (chunk, block) group in the tile grid
    grp_off = np.zeros((NCH, NBLK), np.int64)
    acc = 0
    for c in range(NCH):
        for b in range(NBLK):
            grp_off[c, b] = acc
            acc += tb[c, b]

    idxs_all = np.zeros((NCORES, T * P), np.int64)   # chunk-local table rows
    onehot_all = np.zeros((NCORES, T, P, P), BF16)
    for k in range(NCORES):
        m = core_s == k
        sk, bk, ck, dk = src_s[m], blk_s[m], ch_s[m], dib_s[m]
        off = np.concatenate([[0], np.cumsum(cnt[k].reshape(-1))])
        for c in range(NCH):
            for b in range(NBLK):
                j = c * NBLK + b
                e0, e1 = off[j], off[j + 1]
                n_e = e1 - e0
                if n_e == 0:
                    continue
                slots = grp_off[c, b] * P + np.arange(n_e)
                kk = sk[e0:e1] // NSH
                rr = sk[e0:e1] % NSH
                idxs_all[k, slots] = kk * CHR + (rr - c * CHR)
                tt = slots // P
                ee = slots % P
                onehot_all[k, tt, ee, dk[e0:e1]] = 1.0

    # wrap indices per gather batch; replicate x8 across Q7 cores
    idx_wrapped = np.zeros((NCORES, 16, NB * 64), np.int16)
    for g, (c, s0, nt) in enumerate(batches):
        i0 = s0 * P
        n_i = nt * P
        chunk = idxs_all[:, i0:i0 + n_i].astype(np.int16)
        idx_wrapped[:, :, g * 64: g * 64 + n_i // 16] = (
            chunk.reshape(NCORES, n_i // 16, 16).transpose(0, 2, 1)
        )
    idx_rep = np.tile(idx_wrapped, (1, 8, 1))             # [NC, 128, NB*64]

    # batch-major one-hot layout: oh_b[g, p, t*128+d]
    oh_b = np.zeros((NCORES, NB, P, GB * P), BF16)
    for g, (c, s0, nt) in enumerate(batches):
        chunk = onehot_all[:, s0:s0 + nt]                 # [NC, nt, P, P]
        oh_b[:, g, :, :nt * P] = chunk.transpose(0, 2, 1, 3).reshape(
            NCORES, P, nt * P)

    return dict(
        T=T, NB=NB, tb=tb, batches=batches,
        chunk_of_tile=chunk_of_tile, block_of_tile=block_of_tile,
        first=first, last=last,
        idxs=idx_rep, onehot=oh_b,
    )


def _build_program(ep):
    import concourse.bacc as bacc
    import concourse.mybir as mybir
    import concourse.tile as tile
    from concourse.masks import make_identity

    T, NB = ep["T"], ep["NB"]
    bot, first, last = ep["block_of_tile"], ep["first"], ep["last"]
    cot, tb, batches = ep["chunk_of_tile"], ep["tb"], ep["batches"]
    # last chunk with tiles for each block -> evac trigger
    lgb = [max(c for c in range(NCH) if tb[c, b] > 0) for b in range(NBLK)]
    f32, bf16, i16 = mybir.dt.float32, mybir.dt.bfloat16, mybir.dt.int16

    nc = bacc.Bacc("TRN2", target_bir_lowering=False, debug=False,
                   num_devices=NCORES, num_swdge_queues=4)

    # ---- I/O ----
    xT_in = nc.dram_tensor("xT", [P, NSHP], bf16, kind="ExternalInput")
    w1t_in = nc.dram_tensor("w1t", [P, 512], bf16, kind="ExternalInput")
    wj1_in = nc.dram_tensor("wj1", [P, 4], bf16, kind="ExternalInput")
    w2t_in = nc.dram_tensor("w2t", [P, 4, 512], bf16, kind="ExternalInput")
    wj2_in = nc.dram_tensor("wj2", [P, 4, 4], bf16, kind="ExternalInput")
    w3t_in = nc.dram_tensor("w3t", [P, 4, 100], bf16, kind="ExternalInput")
    wj3_in = nc.dram_tensor("wj3", [P, 4, 1], bf16, kind="ExternalInput")
    oh_in = nc.dram_tensor("onehot", [NB, P, GB * P], bf16, kind="ExternalInput")
    idx_in = nc.dram_tensor("idxs", [P, NB * 64], i16, kind="ExternalInput")
    out_d = nc.dram_tensor("out", [NSH, 100], f32, kind="ExternalOutput")

    # ---- internal DRAM ----
    ag = [nc.dram_tensor(f"ag{i}", [NSHP, L["GW"]], bf16)
          for i, L in enumerate((L1, L2, L3))]
    table = [[nc.dram_tensor(f"table{i}_{c}", [NCORES * CHR, L["GW"]], bf16,
                             addr_space="Shared") for c in range(NCH)]
             for i, L in enumerate((L1, L2, L3))]

    with tile.TileContext(nc, num_cores=NCORES) as tc:
        with (
            tc.tile_pool(name="const", bufs=1) as cp,
            tc.tile_pool(name="sb", bufs=2) as sb,
            tc.tile_pool(name="gat", bufs=6) as gp,
            tc.tile_pool(name="small", bufs=4) as sp,
            tc.tile_pool(name="psum", bufs=2, space="PSUM") as psA,
        ):
            # ---- persistent constants ----
            ident = cp.tile([P, P], bf16, tag="ident")
            make_identity(nc, ident[:])
            idx_sb = cp.tile([P, NB * 64], i16, tag="idx")
            nc.sync.dma_start(out=idx_sb[:], in_=idx_in[:, :])
            w_sb = []
            wj_sb = []
            for i, (L, wt, wj) in enumerate(
                ((L1, w1t_in, wj1_in), (L2, w2t_in, wj2_in), (L3, w3t_in, wj3_in))
            ):
                w = cp.tile([P, L["KB"], L["NOUT"]], bf16, tag=f"w{i}", name=f"w{i}")
                nc.sync.dma_start(out=w[:, 0, :] if i == 0 else w[:],
                                  in_=wt.ap())
                w_sb.append(w)
                wjt = cp.tile([P, L["KB"], L["H"]], bf16, tag=f"wj{i}", name=f"wj{i}")
                nc.sync.dma_start(out=wjt[:, 0, :] if i == 0 else wjt[:],
                                  in_=wj.ap())
                wj_sb.append(wjt)

            xT1 = cp.tile([P, 1, NSHP], bf16, tag="xT1")
            nc.sync.dma_start(out=xT1[:, 0, :], in_=xT_in.ap())
            xT2 = cp.tile([P, 4, NSHP], bf16, tag="xT2", name="xT2")
            xT3 = cp.tile([P, 4, NSHP], bf16, tag="xT3", name="xT3")
            xT_next = [None, xT2, xT3]

            for li, L in enumerate((L1, L2, L3)):
                H, C, KB, GW, UO, NOUT = (L["H"], L["C"], L["KB"], L["GW"],
                                          L["UO"], L["NOUT"])
                xT = xT1 if li == 0 else xT_next[li]
                G_sb = sb.tile([P, RT, GW], bf16, tag="G", bufs=1)

                # ---------- dense + u ----------
                if True:
                    for r in range(RT):
                        ph = psA.tile([P, NOUT], f32, tag="ph")
                        pa = psA.tile([P, H], f32, tag="pa", bufs=1)
                        for kb in range(KB):
                            lhsT = xT[:, kb, r * P:(r + 1) * P]
                            nc.tensor.matmul(ph[:], lhsT, w_sb[li][:, kb, :],
                                             start=(kb == 0), stop=(kb == KB - 1))
                            nc.tensor.matmul(pa[:], lhsT, wj_sb[li][:, kb, :],
                                             start=(kb == 0), stop=(kb == KB - 1))
                        u = sp.tile([P, H], f32, tag="u")
                        nc.scalar.activation(u[:], pa[:],
                                             mybir.ActivationFunctionType.Exp)
                        for h in range(H):
                            nc.vector.tensor_scalar_mul(
                                G_sb[:, r, h * C:(h + 1) * C],
                                ph[:, h * C:(h + 1) * C], u[:, h:h + 1])
                        nc.vector.tensor_copy(out=G_sb[:, r, UO:UO + H], in_=u[:])
                        # stream this row-tile to the AG input now: the
                        # collective then waits on DMA sems, not a serial
                        # chain of 100 compute-writer waits
                        nc.sync.dma_start(
                            out=ag[li][r * P:(r + 1) * P, :],
                            in_=G_sb[:, r, :])

                # fp32 accumulator across source-chunk passes (NCH>1 only)
                if NCH > 1:
                    acc = sb.tile([P, RT, NOUT + H], f32, tag="acc", bufs=1,
                                  name="acc")
                    nc.vector.memset(acc[:], 0.0)

                # ---------- exchange ----------
                for ch in range(NCH):
                    nc.gpsimd.collective_compute(
                        "AllGather", mybir.AluOpType.bypass,
                        replica_groups=[list(range(NCORES))],
                        ins=[ag[li][ch * CHR:(ch + 1) * CHR, :].opt()],
                        outs=[table[li][ch].ap().opt()])

                # ---------- edge aggregation: 4 passes over source chunks -----
                if True:
                    pF = pS = None
                    for g, (ch, s0, nt) in enumerate(batches):
                        gt = gp.tile([P, GB, GW], bf16, tag="gt")
                        nc.gpsimd.dma_gather(
                            gt[:, :nt, :], table[li][ch].ap(),
                            idx_sb[:, g * 64: g * 64 + nt * 8],
                            nt * P, nt * P, GW, queue_num=g % 4)
                        oh = gp.tile([P, GB, P], bf16, tag="oh", bufs=14)
                        nc.sync.dma_start(
                            out=oh[:, :nt, :],
                            in_=oh_in[g, :, :nt * P].rearrange(
                                "p (t d) -> p t d", d=P))
                        for tl in range(nt):
                            ti = s0 + tl
                            b = bot[ti]
                            if first[ti]:
                                pF = psA.tile([P, NOUT], f32, tag="pF")
                                pS = psA.tile([P, H], f32, tag="pS")
                            nc.tensor.matmul(pF[:], oh[:, tl, :],
                                             gt[:, tl, 0:NOUT],
                                             start=bool(first[ti]),
                                             stop=bool(last[ti]))
                            nc.tensor.matmul(pS[:], oh[:, tl, :],
                                             gt[:, tl, UO:UO + H],
                                             start=bool(first[ti]),
                                             stop=bool(last[ti]))
                            if not last[ti]:
                                continue
                            if NCH > 1:
                                # fold this (chunk, block) group into acc
                                nc.vector.tensor_add(
                                    out=acc[:, b, 0:NOUT], in0=acc[:, b, 0:NOUT],
                                    in1=pF[:])
                                nc.vector.tensor_add(
                                    out=acc[:, b, NOUT:NOUT + H],
                                    in0=acc[:, b, NOUT:NOUT + H], in1=pS[:])
                                if cot[ti] != lgb[b]:
                                    continue
                                accF = acc[:, b, 0:NOUT]
                                accS = acc[:, b, NOUT:NOUT + H]
                            else:
                                accF = pF[:]
                                accS = pS[:]
                            # ---------- block evacuation ----------
                            nc.vector.tensor_add(
                                out=accF, in0=accF,
                                in1=G_sb[:, b, 0:NOUT])          # self loop
                            nc.vector.tensor_add(
                                out=accS, in0=accS,
                                in1=G_sb[:, b, UO:UO + H])
                            s1c = sp.tile([P, H], f32, tag="s1c")
                            nc.vector.tensor_scalar_max(s1c[:], accS, 1e-30)
                            rec = sp.tile([P, H], f32, tag="rec")
                            nc.vector.reciprocal(rec[:], s1c[:])
                            if li < 2:
                                ob = sb.tile([P, NOUT], bf16, tag="ob")
                                for h in range(H):
                                    nc.vector.tensor_scalar_mul(
                                        ob[:, h * C:(h + 1) * C],
                                        accF[:, h * C:(h + 1) * C]
                                        if NCH > 1 else pF[:, h * C:(h + 1) * C],
                                        rec[:, h:h + 1])
                                for fb in range(4):
                                    pt = psA.tile([P, P], bf16, tag="pt", bufs=1)
                                    nc.tensor.transpose(
                                        pt[:], ob[:, fb * P:(fb + 1) * P],
                                        ident[:])
                                    nc.vector.tensor_copy(
                                        out=xT_next[li + 1][:, fb,
                                                            b * P:(b + 1) * P],
                                        in_=pt[:])
                            else:
                                o3 = sb.tile([P, 100], f32, tag="o3")
                                nc.vector.tensor_scalar_mul(
                                    o3[:], accF[:, 0:100] if NCH > 1
                                    else pF[:, 0:100], rec[:, 0:1])
                                rows = NSH - b * P if b == NBLK - 1 else P
                                nc.sync.dma_start(
                                    out=out_d[b * P: b * P + rows, :],
                                    in_=o3[:rows, :])
    nc.compile()
    return nc


def _prep_weights(W1, att1, W2, att2, W3, att3):
    """Host-side weight folding and layout prep (fp32 -> bf16)."""
    def fold_wj(W, att, H, C):
        # wj[f_in, h] = sum_c att[h, C+c] * W[h*C+c, f_in]
        return np.stack([att[h, C:] @ W[h * C:(h + 1) * C, :] for h in range(H)],
                        axis=1)  # [F_in, H]

    d = {}
    d["w1t"] = np.ascontiguousarray(W1.T).astype(BF16)               # [128, 512]
    d["wj1"] = fold_wj(W1, att1, 4, 128).astype(BF16)                # [128, 4]
    d["w2t"] = np.ascontiguousarray(W2.T).reshape(4, 128, 512).transpose(
        1, 0, 2).copy().astype(BF16)                                  # [128,4,512]
    d["wj2"] = fold_wj(W2, att2, 4, 128).reshape(4, 128, 4).transpose(
        1, 0, 2).copy().astype(BF16)                                  # [128,4,4]
    d["w3t"] = np.ascontiguousarray(W3.T).reshape(4, 128, 100).transpose(
        1, 0, 2).copy().astype(BF16)                                  # [128,4,100]
    d["wj3"] = fold_wj(W3, att3, 1, 100).reshape(4, 128, 1).transpose(
        1, 0, 2).copy().astype(BF16)                                  # [128,4,1]
    return d


_CACHE = {}


def kernel(x, W1, att1, W2, att2, W3, att3, edge_index):
    from concourse.bass_utils import run_bass_kernel_spmd

    x = np.asarray(x, np.float32)
    edge_index = np.asarray(edge_index).astype(np.int64)

    ep = _preprocess_edges(edge_index)
    wd = _prep_weights(np.asarray(W1, np.float32), np.asarray(att1, np.float32),
                       np.asarray(W2, np.float32), np.asarray(att2, np.float32),
                       np.asarray(W3, np.float32), np.asarray(att3, np.float32))

    key = ("prog", ep["T"], ep["NB"], ep["tb"].tobytes())
    if key not in _CACHE:
        _CACHE[key] = _build_program(ep)
    nc = _CACHE[key]

    in_maps = []
    for k in range(NCORES):
        xk = np.zeros((NSHP, P), np.float32)
        xk[:NSH] = x[k * NSH:(k + 1) * NSH]
        m = dict(wd)
        m["xT"] = np.ascontiguousarray(xk.T).astype(BF16)
        m["onehot"] = ep["onehot"][k]
        m["idxs"] = ep["idxs"][k]
        in_maps.append(m)

    res = run_bass_kernel_spmd(nc, in_maps, core_ids=list(range(NCORES)))
    out = np.concatenate([res.results[k]["out"] for k in range(NCORES)], axis=0)
    return out.astype(np.float32)


def kernel_traced(inputs):
    """test-harness entry: returns (out, BassKernelResults with trace)."""
    from concourse.bass_utils import run_bass_kernel_spmd

    x = np.asarray(inputs["x"], np.float32)
    edge_index = np.asarray(inputs["edge_index"]).astype(np.int64)
    ep = _preprocess_edges(edge_index)
    wd = _prep_weights(*[np.asarray(inputs[k], np.float32) for k in
                         ("W1", "att1", "W2", "att2", "W3", "att3")])
    key = ("prog", ep["T"], ep["NB"], ep["tb"].tobytes())
    if key not in _CACHE:
        _CACHE[key] = _build_program(ep)
    nc = _CACHE[key]
    in_maps = []
    for k in range(NCORES):
        xk = np.zeros((NSHP, P), np.float32)
        xk[:NSH] = x[k * NSH:(k + 1) * NSH]
        m = dict(wd)
        m["xT"] = np.ascontiguousarray(xk.T).astype(BF16)
        m["onehot"] = ep["onehot"][k]
        m["idxs"] = ep["idxs"][k]
        in_maps.append(m)
    res = run_bass_kernel_spmd(nc, in_maps, core_ids=list(range(NCORES)),
                               trace=True)
    out = np.concatenate([res.results[k]["out"] for k in range(NCORES)], axis=0)
    return out.astype(np.float32), res


# revision 19
# speedup vs baseline: 1.0108x; 1.0108x over previous
"""GAT (3-layer, no-LeakyReLU) on 8 Trainium2 NeuronCores.

Math: the reference omits LeakyReLU on attention logits, so softmax is
separable: with aj[n,h] = <h[n,h,:], att[h, C:]> and u = exp(aj),
    out[d] = sum_{e: dst=d} u[src_e] * h[src_e] / sum_e u[src_e]
(the ai[dst] term cancels inside the per-destination softmax).

Per layer, per core (nodes sharded by destination, graph structure
preprocessed on host):
  1. dense:  h = x @ W.T (rows layout, lhsT = transposed input xT)
             aj = x @ wj  (wj = W.T @ att_j, host-folded), u = exp(aj)
  2. G = [u*h | u | pad] bf16, AllGather -> replicated table in HBM
  3. per dst-block of 128 nodes: gather G[src] rows (dma_gather),
     one-hot matmul accumulates F = sum onehot.T @ g  and S1 = sum u
  4. out = F / S1 ; transpose to xT for the next layer's dense.
"""

import numpy as np
import ml_dtypes

N = 20000
E = 320000
NCORES = 8
NSH = 2500            # nodes per core (true)
NSHP = 2560           # padded to 20 x 128
NPAD = NCORES * NSHP  # padded table rows
P = 128
RT = NSHP // P        # row tiles per core = 20
NBLK = RT             # dst blocks per core = 20
GB = 8                # gather batch: tiles per dma_gather

# layer configs: (heads, C_out, KB in-blocks, G width, u col offset)
L1 = dict(H=4, C=128, KB=1, GW=640, UO=512, NOUT=512)
L2 = dict(H=4, C=128, KB=4, GW=640, UO=512, NOUT=512)
L3 = dict(H=1, C=100, KB=4, GW=256, UO=100, NOUT=100)

BF16 = ml_dtypes.bfloat16


NCH = 1               # source chunks (table split for AG/gather overlap)
CHR = NSHP // NCH     # 640 local rows per chunk


def _preprocess_edges(edge_index):
    """Sort edges by (dst core, src chunk, dst block); equalize per
    (chunk, block) tile counts across cores; build per-core gather
    indices (chunk-local) + one-hot tiles. Tiles are ordered pass-major
    (chunk outer, block inner) and gather batches never cross a chunk
    boundary, so pass-c gathers only need AG chunk c."""
    src = edge_index[0].astype(np.int64)
    dst = edge_index[1].astype(np.int64)
    core = dst // NSH
    d_loc = dst % NSH
    blk = d_loc // P
    d_in_blk = d_loc % P
    s_loc = src % NSH
    s_chunk = s_loc // CHR       # 0..3 (s_loc < 2500 < 2560)

    order = np.lexsort((src, blk, s_chunk, core))
    src_s = src[order]
    core_s = core[order]
    blk_s = blk[order]
    dib_s = d_in_blk[order]
    ch_s = s_chunk[order]

    # per (core, chunk, block) counts
    cnt = np.zeros((NCORES, NCH, NBLK), np.int64)
    for k in range(NCORES):
        m = core_s == k
        idx2 = ch_s[m] * NBLK + blk_s[m]
        cnt[k] = np.bincount(idx2, minlength=NCH * NBLK).reshape(NCH, NBLK)

    tb = np.ceil(cnt / P).astype(np.int64).max(axis=0)    # [NCH, NBLK]
    tch = tb.sum(axis=1)                                  # tiles per chunk
    T = int(tb.sum())
    nb_ch = [(int(t) + GB - 1) // GB for t in tch]        # batches per chunk
    NB = int(sum(nb_ch))

    # tile grid: pass-major (chunk, block, tile)
    chunk_of_tile = []
    block_of_tile = []
    first = []
    last = []
    for c in range(NCH):
        for b in range(NBLK):
            for t in range(tb[c, b]):
                chunk_of_tile.append(c)
                block_of_tile.append(b)
                first.append(t == 0)
                last.append(t == tb[c, b] - 1)
    chunk_of_tile = np.array(chunk_of_tile)
    block_of_tile = np.array(block_of_tile)
    first = np.array(first)
    last = np.array(last)

    # batch layout: per chunk, batches of GB tiles (last partial)
    batches = []   # (chunk, tile_start, n_tiles)
    tile_base = np.concatenate([[0], np.cumsum(tch)])
    for c in range(NCH):
        t0 = int(tile_base[c])
        for g in range(nb_ch[c]):
            s0 = t0 + g * GB
            batches.append((c, s0, min(GB, t0 + int(tch[c]) - s0)))

    # slot offsets of each (chunk, block) group in the tile grid
    grp_off = np.zeros((NCH, NBLK), np.int64)
    acc = 0
    for c in range(NCH):
        for b in range(NBLK):
            grp_off[c, b] = acc
            acc += tb[c, b]

    idxs_all = np.zeros((NCORES, T * P), np.int64)   # chunk-local table rows
    onehot_all = np.zeros((NCORES, T, P, P), BF16)
    for k in range(NCORES):
        m = core_s == k
        sk, bk, ck, dk = src_s[m], blk_s[m], ch_s[m], dib_s[m]
        off = np.concatenate([[0], np.cumsum(cnt[k].reshape(-1))])
        for c in range(NCH):
            for b in range(NBLK):
                j = c * NBLK + b
                e0, e1 = off[j], off[j + 1]
                n_e = e1 - e0
                if n_e == 0:
                    continue
                slots = grp_off[c, b] * P + np.arange(n_e)
                kk = sk[e0:e1] // NSH
                rr = sk[e0:e1] % NSH
                idxs_all[k, slots] = kk * CHR + (rr - c * CHR)
                tt = slots // P
                ee = slots % P
                onehot_all[k, tt, ee, dk[e0:e1]] = 1.0

    # wrap indices per gather batch; replicate x8 across Q7 cores
    idx_wrapped = np.zeros((NCORES, 16, NB * 64), np.int16)
    for g, (c, s0, nt) in enumerate(batches):
        i0 = s0 * P
        n_i = nt * P
        chunk = idxs_all[:, i0:i0 + n_i].astype(np.int16)
        idx_wrapped[:, :, g * 64: g * 64 + n_i // 16] = (
            chunk.reshape(NCORES, n_i // 16, 16).transpose(0, 2, 1)
        )
    idx_rep = np.tile(idx_wrapped, (1, 8, 1))             # [NC, 128, NB*64]

    # batch-major one-hot layout: oh_b[g, p, t*128+d]
    oh_b = np.zeros((NCORES, NB, P, GB * P), BF16)
    for g, (c, s0, nt) in enumerate(batches):
        chunk = onehot_all[:, s0:s0 + nt]                 # [NC, nt, P, P]
        oh_b[:, g, :, :nt * P] = chunk.transpose(0, 2, 1, 3).reshape(
            NCORES, P, nt * P)

    return dict(
        T=T, NB=NB, tb=tb, batches=batches,
        chunk_of_tile=chunk_of_tile, block_of_tile=block_of_tile,
        first=first, last=last,
        idxs=idx_rep, onehot=oh_b,
    )


def _build_program(ep):
    import concourse.bacc as bacc
    import concourse.mybir as mybir
    import concourse.tile as tile
    from concourse.masks import make_identity

    T, NB = ep["T"], ep["NB"]
    bot, first, last = ep["block_of_tile"], ep["first"], ep["last"]
    cot, tb, batches = ep["chunk_of_tile"], ep["tb"], ep["batches"]
    # last chunk with tiles for each block -> evac trigger
    lgb = [max(c for c in range(NCH) if tb[c, b] > 0) for b in range(NBLK)]
    f32, bf16, i16 = mybir.dt.float32, mybir.dt.bfloat16, mybir.dt.int16

    nc = bacc.Bacc("TRN2", target_bir_lowering=False, debug=False,
                   num_devices=NCORES, num_swdge_queues=4)

    # ---- I/O ----
    xT_in = nc.dram_tensor("xT", [P, NSHP], bf16, kind="ExternalInput")
    w1t_in = nc.dram_tensor("w1t", [P, 512], bf16, kind="ExternalInput")
    wj1_in = nc.dram_tensor("wj1", [P, 4], bf16, kind="ExternalInput")
    w2t_in = nc.dram_tensor("w2t", [P, 4, 512], bf16, kind="ExternalInput")
    wj2_in = nc.dram_tensor("wj2", [P, 4, 4], bf16, kind="ExternalInput")
    w3t_in = nc.dram_tensor("w3t", [P, 4, 100], bf16, kind="ExternalInput")
    wj3_in = nc.dram_tensor("wj3", [P, 4, 1], bf16, kind="ExternalInput")
    oh_in = nc.dram_tensor("onehot", [NB, P, GB * P], bf16, kind="ExternalInput")
    idx_in = nc.dram_tensor("idxs", [P, NB * 64], i16, kind="ExternalInput")
    out_d = nc.dram_tensor("out", [NSH, 100], f32, kind="ExternalOutput")

    # ---- internal DRAM ----
    ag = [nc.dram_tensor(f"ag{i}", [NSHP, L["GW"]], bf16)
          for i, L in enumerate((L1, L2, L3))]
    table = [[nc.dram_tensor(f"table{i}_{c}", [NCORES * CHR, L["GW"]], bf16,
                             addr_space="Shared") for c in range(NCH)]
             for i, L in enumerate((L1, L2, L3))]

    with tile.TileContext(nc, num_cores=NCORES) as tc:
        with (
            tc.tile_pool(name="const", bufs=1) as cp,
            tc.tile_pool(name="sb", bufs=2) as sb,
            tc.tile_pool(name="gat", bufs=6) as gp,
            tc.tile_pool(name="small", bufs=4) as sp,
            tc.tile_pool(name="psum", bufs=2, space="PSUM") as psA,
        ):
            # ---- persistent constants ----
            ident = cp.tile([P, P], bf16, tag="ident")
            make_identity(nc, ident[:])
            idx_sb = cp.tile([P, NB * 64], i16, tag="idx")
            nc.sync.dma_start(out=idx_sb[:], in_=idx_in[:, :])
            w_sb = []
            wj_sb = []
            for i, (L, wt, wj) in enumerate(
                ((L1, w1t_in, wj1_in), (L2, w2t_in, wj2_in), (L3, w3t_in, wj3_in))
            ):
                w = cp.tile([P, L["KB"], L["NOUT"]], bf16, tag=f"w{i}", name=f"w{i}")
                nc.sync.dma_start(out=w[:, 0, :] if i == 0 else w[:],
                                  in_=wt.ap())
                w_sb.append(w)
                wjt = cp.tile([P, L["KB"], L["H"]], bf16, tag=f"wj{i}", name=f"wj{i}")
                nc.sync.dma_start(out=wjt[:, 0, :] if i == 0 else wjt[:],
                                  in_=wj.ap())
                wj_sb.append(wjt)

            xT1 = cp.tile([P, 1, NSHP], bf16, tag="xT1")
            nc.sync.dma_start(out=xT1[:, 0, :], in_=xT_in.ap())
            xT2 = cp.tile([P, 4, NSHP], bf16, tag="xT2", name="xT2")
            xT3 = cp.tile([P, 4, NSHP], bf16, tag="xT3", name="xT3")
            xT_next = [None, xT2, xT3]

            for li, L in enumerate((L1, L2, L3)):
                H, C, KB, GW, UO, NOUT = (L["H"], L["C"], L["KB"], L["GW"],
                                          L["UO"], L["NOUT"])
                xT = xT1 if li == 0 else xT_next[li]
                G_sb = sb.tile([P, RT, GW], bf16, tag="G", bufs=1)

                # ---------- dense + u ----------
                if True:
                    for r in range(RT):
                        ph = psA.tile([P, NOUT], f32, tag="ph")
                        pa = psA.tile([P, H], f32, tag="pa", bufs=1)
                        for kb in range(KB):
                            lhsT = xT[:, kb, r * P:(r + 1) * P]
                            nc.tensor.matmul(ph[:], lhsT, w_sb[li][:, kb, :],
                                             start=(kb == 0), stop=(kb == KB - 1))
                            nc.tensor.matmul(pa[:], lhsT, wj_sb[li][:, kb, :],
                                             start=(kb == 0), stop=(kb == KB - 1))
                        u = sp.tile([P, H], f32, tag="u")
                        nc.scalar.activation(u[:], pa[:],
                                             mybir.ActivationFunctionType.Exp)
                        for h in range(H):
                            nc.vector.tensor_scalar_mul(
                                G_sb[:, r, h * C:(h + 1) * C],
                                ph[:, h * C:(h + 1) * C], u[:, h:h + 1])
                        nc.vector.tensor_copy(out=G_sb[:, r, UO:UO + H], in_=u[:])
                        # stream this row-tile to the AG input now: the
                        # collective then waits on DMA sems, not a serial
                        # chain of 100 compute-writer waits
                        nc.sync.dma_start(
                            out=ag[li][r * P:(r + 1) * P, :],
                            in_=G_sb[:, r, :])

                # fp32 accumulator across source-chunk passes (NCH>1 only)
                if NCH > 1:
                    acc = sb.tile([P, RT, NOUT + H], f32, tag="acc", bufs=1,
                                  name="acc")
                    nc.vector.memset(acc[:], 0.0)

                # ---------- exchange ----------
                for ch in range(NCH):
                    nc.gpsimd.collective_compute(
                        "AllGather", mybir.AluOpType.bypass,
                        replica_groups=[list(range(NCORES))],
                        ins=[ag[li][ch * CHR:(ch + 1) * CHR, :].opt()],
                        outs=[table[li][ch].ap().opt()])

                # ---------- edge aggregation: 4 passes over source chunks -----
                if True:
                    pF = pS = None
                    for g, (ch, s0, nt) in enumerate(batches):
                        gt = gp.tile([P, GB, GW], bf16, tag="gt")
                        nc.gpsimd.dma_gather(
                            gt[:, :nt, :], table[li][ch].ap(),
                            idx_sb[:, g * 64: g * 64 + nt * 8],
                            nt * P, nt * P, GW, queue_num=g % 4)
                        oh = gp.tile([P, GB, P], bf16, tag="oh", bufs=14)
                        nc.sync.dma_start(
                            out=oh[:, :nt, :],
                            in_=oh_in[g, :, :nt * P].rearrange(
                                "p (t d) -> p t d", d=P))
                        for tl in range(nt):
                            ti = s0 + tl
                            b = bot[ti]
                            if first[ti]:
                                pF = psA.tile([P, NOUT], f32, tag="pF")
                                pS = psA.tile([P, H], f32, tag="pS")
                            nc.tensor.matmul(pF[:], oh[:, tl, :],
                                             gt[:, tl, 0:NOUT],
                                             start=bool(first[ti]),
                                             stop=bool(last[ti]))
                            nc.tensor.matmul(pS[:], oh[:, tl, :],
                                             gt[:, tl, UO:UO + H],
                                             start=bool(first[ti]),
                                             stop=bool(last[ti]))
                            if not last[ti]:
                                continue
                            if NCH > 1:
                                # fold this (chunk, block) group into acc
                                nc.vector.tensor_add(
                                    out=acc[:, b, 0:NOUT], in0=acc[:, b, 0:NOUT],
                                    in1=pF[:])
                                nc.vector.tensor_add(
                                    out=acc[:, b, NOUT:NOUT + H],
                                    in0=acc[:, b, NOUT:NOUT + H], in1=pS[:])
                                if cot[ti] != lgb[b]:
                                    continue
                                accF = acc[:, b, 0:NOUT]
                                accS = acc[:, b, NOUT:NOUT + H]
                            else:
                                accF = pF[:]
                                accS = pS[:]
                            # ---------- block evacuation ----------
                            nc.vector.tensor_add(
                                out=accF, in0=accF,
                                in1=G_sb[:, b, 0:NOUT])          # self loop
                            nc.vector.tensor_add(
                                out=accS, in0=accS,
                                in1=G_sb[:, b, UO:UO + H])
                            s1c = sp.tile([P, H], f32, tag="s1c")
                            nc.vector.tensor_scalar_max(s1c[:], accS, 1e-30)
                            rec = sp.tile([P, H], f32, tag="rec")
                            nc.vector.reciprocal(rec[:], s1c[:])
                            if li < 2:
                                ob = sb.tile([P, NOUT], bf16, tag="ob")
                                for h in range(H):
                                    nc.vector.tensor_scalar_mul(
                                        ob[:, h * C:(h + 1) * C],
                                        accF[:, h * C:(h + 1) * C]
                                        if NCH > 1 else pF[:, h * C:(h + 1) * C],
                                        rec[:, h:h + 1])
                                for fb in range(4):
                                    pt = psA.tile([P, P], bf16, tag="pt", bufs=1)
                                    nc.tensor.transpose(
                                        pt[:], ob[:, fb * P:(fb + 1) * P],
                                        ident[:])
                                    nc.vector.tensor_copy(
                                        out=xT_next[li + 1][:, fb,
                                                            b * P:(b + 1) * P],
                                        in_=pt[:])
                            else:
                                o3 = sb.tile([P, 100], f32, tag="o3")
                                nc.vector.tensor_scalar_mul(
                                    o3[:], accF[:, 0:100] if NCH > 1
                                    else pF[:, 0:100], rec[:, 0:1])
                                rows = NSH - b * P if b == NBLK - 1 else P
                                nc.sync.dma_start(
                                    out=out_d[b * P: b * P + rows, :],
                                    in_=o3[:rows, :])
    nc.compile()
    return nc


def _prep_weights(W1, att1, W2, att2, W3, att3):
    """Host-side weight folding and layout prep (fp32 -> bf16)."""
    def fold_wj(W, att, H, C):
        # wj[f_in, h] = sum_c att[h, C+c] * W[h*C+c, f_in]
        return np.stack([att[h, C:] @ W[h * C:(h + 1) * C, :] for h in range(H)],
                        axis=1)  # [F_in, H]

    d = {}
    d["w1t"] = np.ascontiguousarray(W1.T).astype(BF16)               # [128, 512]
    d["wj1"] = fold_wj(W1, att1, 4, 128).astype(BF16)                # [128, 4]
    d["w2t"] = np.ascontiguousarray(W2.T).reshape(4, 128, 512).transpose(
        1, 0, 2).copy().astype(BF16)                                  # [128,4,512]
    d["wj2"] = fold_wj(W2, att2, 4, 128).reshape(4, 128, 4).transpose(
        1, 0, 2).copy().astype(BF16)                                  # [128,4,4]
    d["w3t"] = np.ascontiguousarray(W3.T).reshape(4, 128, 100).transpose(
        1, 0, 2).copy().astype(BF16)                                  # [128,4,100]
    d["wj3"] = fold_wj(W3, att3, 1, 100).reshape(4, 128, 1).transpose(
        1, 0, 2).copy().astype(BF16)                                  # [128,4,1]
    return d


_CACHE = {}


def kernel(x, W1, att1, W2, att2, W3, att3, edge_index):
    from concourse.bass_utils import run_bass_kernel_spmd

    x = np.asarray(x, np.float32)
    edge_index = np.asarray(edge_index).astype(np.int64)

    ep = _preprocess_edges(edge_index)
    wd = _prep_weights(np.asarray(W1, np.float32), np.asarray(att1, np.float32),
                       np.asarray(W2, np.float32), np.asarray(att2, np.float32),
                       np.asarray(W3, np.float32), np.asarray(att3, np.float32))

    key = ("prog", ep["T"], ep["NB"], ep["tb"].tobytes())
    if key not in _CACHE:
        _CACHE[key] = _build_program(ep)
    nc = _CACHE[key]

    in_maps = []
    for k in range(NCORES):
        xk = np.zeros((NSHP, P), np.float32)
        xk[:NSH] = x[k * NSH:(k + 1) * NSH]
        m = dict(wd)
        m["xT"] = np.ascontiguousarray(xk.T).astype(BF16)
        m["onehot"] = ep["onehot"][k]
        m["idxs"] = ep["idxs"][k]
        in_maps.append(m)

    res = run_bass_kernel_spmd(nc, in_maps, core_ids=list(range(NCORES)))
    out = np.concatenate([res.results[k]["out"] for k in range(NCORES)], axis=0)
    return out.astype(np.float32)


def kernel_traced(inputs):
    """test-harness entry: returns (out, BassKernelResults with trace)."""
    from concourse.bass_utils import run_bass_kernel_spmd

    x = np.asarray(inputs["x"], np.float32)
    edge_index = np.asarray(inputs["edge_index"]).astype(np.int64)
    ep = _preprocess_edges(edge_index)
    wd = _prep_weights(*[np.asarray(inputs[k], np.float32) for k in
                         ("W1", "att1", "W2", "att2", "W3", "att3")])
    key = ("prog", ep["T"], ep["NB"], ep["tb"].tobytes())
    if key not in _CACHE:
        _CACHE[key] = _build_program(ep)
    nc = _CACHE[key]
    in_maps = []
    for k in range(NCORES):
        xk = np.zeros((NSHP, P), np.float32)
        xk[:NSH] = x[k * NSH:(k + 1) * NSH]
        m = dict(wd)
        m["xT"] = np.ascontiguousarray(xk.T).astype(BF16)
        m["onehot"] = ep["onehot"][k]
        m["idxs"] = ep["idxs"][k]
        in_maps.append(m)
    res = run_bass_kernel_spmd(nc, in_maps, core_ids=list(range(NCORES)),
                               trace=True)
    out = np.concatenate([res.results[k]["out"] for k in range(NCORES)], axis=0)
    return out.astype(np.float32), res
